# revision 5
# baseline (speedup 1.0000x reference)
"""AttnBlock (GroupNorm + single-head spatial attention + proj + residual)
for Trainium2, SPMD across 8 NeuronCores.

Sharding: data-parallel over batch (4 images) x 2-way split of query
positions per image => 8 cores.  Attention is computed per-image with the
full key/value set on every core, so there are no collectives.

Per-core algorithm (image b, query half h):
  - Spatial positions of the local image copy are rolled so the core's
    2048 query positions are always local positions [0, 2048).  Attention
    and GroupNorm are permutation-invariant over spatial positions, so the
    roll is transparent; the host un-rolls when assembling the output.
  - x is loaded in bf16 (half the HBM traffic of f32); GroupNorm stats on
    bf16 x are within ~1e-5 of exact.  GroupNorm is folded into the
    projections: h = a*x + b, with a folded into the (bf16) weights.
  - wproj is folded into the v projection on the host (softmax rows sum
    to one); the remaining per-channel output constant (w2@b + wproj@bv
    + bproj) is shipped back as a tiny `b2o` output and added on the
    host, together with the residual x.
  - k's projection bias is dropped (adds a softmax-invariant constant).
  - Attention runs in fp8(e4m3) with DoubleRow matmuls (2 fp8 values per
    PE cell => one matmul contracts 256): q/k are packed [128, 2(ch), .],
    so each 128-key score tile is a single matmul; exp is applied to two
    score tiles at once (one 1024-wide ACT op) with a constant shift
    (softmax-invariant, keeps e^s inside fp8 range), writing the packed
    [128, 2(jtile), 512] fp8 moving operand for PV.  PV is computed with
    v^T stationary ([128 j, 2, 128 ch] fp8 pairs) accumulating
    aT[ch, i-block]; an all-ones fp8 stationary produces the softmax
    denominator broadcast across all 128 partitions for free.
  - PE warm-up matmuls fill the initial DMA/stats wait so the HAM clock
    gate is released before the real matmul stream starts.
"""

import numpy as np
import ml_dtypes

import concourse.bacc as bacc
import concourse.bass as bass
import concourse.mybir as mybir
import concourse.tile as tile
from concourse.bass_utils import run_bass_kernel_spmd

F32 = mybir.dt.float32
BF16 = mybir.dt.bfloat16
FP8 = mybir.dt.float8e4
DR = mybir.MatmulPerfMode.DoubleRow

C = 256          # channels
HW = 4096        # spatial positions (64*64)
B = 4            # batch
NCORES = 8
IH = HW // 2     # query positions per core
P = 128          # partitions
NCC = C // P     # channel chunks (2)
IBLK = 512       # query i-block
NIB = IH // IBLK # 4 i-blocks per core
NJT = HW // P    # 32 key tiles
NPAIR = NJT // 2 # 16 key-tile pairs (DoubleRow)
EPS = 1e-6
SCALE = 1.0 / 16.0  # 1/sqrt(C)
SHIFT = 3.0         # exp(s*SCALE - SHIFT): keeps e^s in fp8 e4m3 range

NPBF16 = ml_dtypes.bfloat16

_PROGRAM = None  # cached (nc)
LAST_RESULTS = None  # BassKernelResults of the most recent run (for test harness)
TRACE = False


def _build_program():
    nc = bacc.Bacc()

    xr_d = nc.declare_dram_parameter("xr", [C, HW], BF16, isOutput=False)
    wq_d = nc.declare_dram_parameter("wqt", [C, C], BF16, isOutput=False)
    wk_d = nc.declare_dram_parameter("wkt", [C, C], BF16, isOutput=False)
    w2_d = nc.declare_dram_parameter("w2t", [C, C], BF16, isOutput=False)
    bq_d = nc.declare_dram_parameter("bq", [C], F32, isOutput=False)
    b2h_d = nc.declare_dram_parameter("b2h", [C], F32, isOutput=False)  # wproj@bv+bproj
    gns_d = nc.declare_dram_parameter("gns", [C], F32, isOutput=False)
    gnb_d = nc.declare_dram_parameter("gnb", [C], F32, isOutput=False)
    out_d = nc.declare_dram_parameter("out", [C, IH], F32, isOutput=True)
    b2o_d = nc.declare_dram_parameter("b2o", [C], F32, isOutput=True)

    with tile.TileContext(nc) as tc:
        with (
            tc.tile_pool(name="wt", bufs=1) as wt,
            tc.tile_pool(name="xp", bufs=1) as xp,
            tc.tile_pool(name="qkv", bufs=1) as qkv,
            tc.tile_pool(name="scr", bufs=2) as scr,
        ):
            # ---------- constants ----------
            G = wt.tile([P, P], F32, tag="G", name="G")
            nc.gpsimd.memset(G, 0.0)
            nc.gpsimd.memset(G[0:64, 0:64], 1.0 / 64.0)
            nc.gpsimd.memset(G[64:128, 64:128], 1.0 / 64.0)
            eps_t = wt.tile([P, 1], F32, tag="eps", name="eps")
            nc.vector.memset(eps_t, EPS)
            shift_t = wt.tile([P, 1], F32, tag="shift", name="shift")
            nc.vector.memset(shift_t, -SHIFT)
            ones_pair = wt.tile([P, 2, P], FP8, tag="ones8", name="ones8")
            nc.vector.memset(ones_pair, 1.0)
            ones_row = wt.tile([1, P], BF16, tag="ones_row", name="ones_row")
            nc.vector.memset(ones_row, 1.0)

            # ---------- x loads first (startup critical path) ----------
            # issue from multiple sequencers: one dma_start costs ~0.6us of
            # sequencer issue time, and x is the critical path
            xr_sb = [xp.tile([P, HW], BF16, tag=f"xr{cc}", name=f"xr{cc}")
                     for cc in range(NCC)]
            _eng = [nc.sync, nc.scalar, nc.gpsimd]
            for w in range(8):
                for cc in range(NCC):
                    _eng[(w * NCC + cc) % 3].dma_start(
                        out=xr_sb[cc][:, w * 512:(w + 1) * 512],
                        in_=xr_d[cc * P:(cc + 1) * P, w * 512:(w + 1) * 512],
                    )

            # ---------- load weights / params ----------
            w_sb = {}
            for name, d in (("q", wq_d), ("k", wk_d), ("v", w2_d)):
                for cc in range(NCC):
                    t = wt.tile([P, C], BF16, tag=f"w{name}{cc}", name=f"w{name}{cc}")
                    nc.scalar.dma_start(out=t, in_=d[cc * P:(cc + 1) * P, :])
                    w_sb[name, cc] = t
            par_sb = {}
            for name, d in (("bq", bq_d), ("gns", gns_d), ("gnb", gnb_d)):
                for cc in range(NCC):
                    t = wt.tile([P, 1], F32, tag=f"{name}{cc}", name=f"{name}{cc}")
                    nc.scalar.dma_start(out=t, in_=d[cc * P:(cc + 1) * P].unsqueeze(1))
                    par_sb[name, cc] = t
            b2h_sb = wt.tile([1, C], F32, tag="b2h", name="b2h")
            nc.sync.dma_start(out=b2h_sb, in_=b2h_d[:].unsqueeze(0))

            # ---------- GroupNorm stats (on bf16 x; error ~1e-5) ----------
            with tc.tile_pool(name="psA", bufs=2, space="PSUM") as psA:
                # PE warm-up while x DMA + stats run: fills idle time and
                # brings HAM out of the cold 1.2 GHz state before real work
                warm_ps = psA.tile([P, 128], F32, tag="warm", name="warm")
                warm_rhs = wt.tile([P, 128], F32, tag="warm_rhs", name="warm_rhs")
                nc.gpsimd.memset(warm_rhs, 0.0)
                for _ in range(26):
                    nc.tensor.matmul(warm_ps, G, warm_rhs, start=True, stop=True)
                a_sb, b_sb = [], []
                st6s = [scr.tile([P, 8, 6], F32, tag=f"st6{cc}", name=f"st6{cc}")
                        for cc in range(NCC)]
                for w in range(8):
                    for cc in range(NCC):
                        nc.vector.bn_stats(out=st6s[cc][:, w, :], in_=xr_sb[cc][:, w * 512:(w + 1) * 512])
                for cc in range(NCC):
                    st6 = st6s[cc]
                    mv = scr.tile([P, 2], F32, tag="mv", name="mv")
                    nc.vector.bn_aggr(out=mv, in_=st6)
                    st3 = scr.tile([P, 3], F32, tag="st3", name="st3")
                    nc.vector.tensor_copy(st3[:, 0:2], mv)
                    nc.vector.tensor_mul(st3[:, 2:3], mv[:, 0:1], mv[:, 0:1])
                    gp = psA.tile([P, 3], F32, tag="gp", name="gp")
                    nc.tensor.matmul(gp, G, st3, start=True, stop=True)
                    # group stats, broadcast per channel: mean, E[var], E[mean^2]
                    gs = scr.tile([P, 3], F32, tag="gs", name="gs")
                    nc.vector.tensor_copy(gs, gp)
                    t1 = scr.tile([P, 1], F32, tag="t1", name="t1")
                    nc.vector.tensor_mul(t1, gs[:, 0:1], gs[:, 0:1])
                    vg = scr.tile([P, 1], F32, tag="vg", name="vg")
                    nc.vector.tensor_add(vg, gs[:, 1:2], gs[:, 2:3])
                    nc.vector.tensor_sub(vg, vg, t1)
                    sd = scr.tile([P, 1], F32, tag="sd", name="sd")
                    nc.scalar.activation(out=sd, in_=vg, func=mybir.ActivationFunctionType.Sqrt, bias=eps_t)
                    rstd = scr.tile([P, 1], F32, tag="rstd", name="rstd")
                    nc.vector.reciprocal(rstd, sd)
                    a_t = wt.tile([P, 1], F32, tag=f"a{cc}", name=f"a{cc}")
                    nc.vector.tensor_mul(a_t, rstd, par_sb["gns", cc])
                    t2 = scr.tile([P, 1], F32, tag="t2", name="t2")
                    nc.vector.tensor_mul(t2, gs[:, 0:1], a_t)
                    b_t = wt.tile([P, 1], BF16, tag=f"b{cc}", name=f"b{cc}")
                    nc.vector.tensor_sub(b_t, par_sb["gnb", cc], t2)
                    a_sb.append(a_t)
                    b_sb.append(b_t)

                for _ in range(14):
                    nc.tensor.matmul(warm_ps, G, warm_rhs, start=True, stop=True)

                # ---------- fold GroupNorm scale into weights ----------
                wf = {}
                for name in ("q", "k", "v"):
                    for cc in range(NCC):
                        t = wt.tile([P, C], BF16, tag=f"wf{name}{cc}", name=f"wf{name}{cc}")
                        nc.vector.tensor_scalar_mul(t, w_sb[name, cc], a_sb[cc])
                        wf[name, cc] = t

                # ---------- effective biases ----------
                be = {}
                for cc in range(NCC):
                    bp = psA.tile([P, 1], F32, tag="bp", name="bp")
                    nc.tensor.matmul(bp, w_sb["q", 0][:, cc * P:(cc + 1) * P], b_sb[0], start=True, stop=False)
                    nc.tensor.matmul(bp, w_sb["q", 1][:, cc * P:(cc + 1) * P], b_sb[1], start=False, stop=True)
                    t = wt.tile([P, 1], F32, tag=f"beq{cc}", name=f"beq{cc}")
                    nc.vector.tensor_add(t, bp, par_sb["bq", cc])
                    be["q", cc] = t
                # per-channel output constant w2@b + (wproj@bv + bproj),
                # shipped to the host (softmax rows sum to 1, so it can be
                # added after attention)
                b2p = psA.tile([1, C], F32, tag="b2p", name="b2p")
                nc.tensor.matmul(b2p, b_sb[0], w_sb["v", 0], start=True, stop=False)
                nc.tensor.matmul(b2p, b_sb[1], w_sb["v", 1], start=False, stop=True)
                b2row = wt.tile([1, C], F32, tag="b2row", name="b2row")
                nc.vector.tensor_add(b2row, b2p, b2h_sb)
                nc.sync.dma_start(out=b2o_d[:].unsqueeze(0), in_=b2row)

            # ---------- projections (bf16 in, fp8 packed out) ----------
            # q packed [128, 2(ch chunk), IH]; k packed [128, jt, 2(ch), 128];
            # vT packed [128, jt-pair, 2(jt), C]
            q_sb = qkv.tile([P, NCC, IH], FP8, tag="q8", name="q8")
            k_sb = qkv.tile([P, NJT, NCC, P], FP8, tag="k8", name="k8")
            vT_sb = qkv.tile([P, NPAIR, 2, C], FP8, tag="vT8", name="vT8")

            with tc.tile_pool(name="psB", bufs=3, space="PSUM") as psB:
                for cc in range(NCC):
                    for ib in range(NIB):
                        pq = psB.tile([P, IBLK], F32, tag="pq", name="pq")
                        sl = slice(ib * IBLK, (ib + 1) * IBLK)
                        nc.tensor.matmul(pq, wf["q", 0][:, cc * P:(cc + 1) * P], xr_sb[0][:, sl], start=True, stop=False)
                        nc.tensor.matmul(pq, wf["q", 1][:, cc * P:(cc + 1) * P], xr_sb[1][:, sl], start=False, stop=True)
                        nc.vector.tensor_scalar_add(q_sb[:, cc, sl], pq, be["q", cc])
                for cc in range(NCC):
                    for jb in range(HW // IBLK):
                        pk = psB.tile([P, IBLK], F32, tag="pq", name="pq")
                        sl = slice(jb * IBLK, (jb + 1) * IBLK)
                        nc.tensor.matmul(pk, wf["k", 0][:, cc * P:(cc + 1) * P], xr_sb[0][:, sl], start=True, stop=False)
                        nc.tensor.matmul(pk, wf["k", 1][:, cc * P:(cc + 1) * P], xr_sb[1][:, sl], start=False, stop=True)
                        # k's bias only adds a j-constant to each softmax row
                        # (q_i . bke), so it is dropped; fp8 pack on ACT
                        nc.scalar.copy(k_sb[:, 4 * jb:4 * jb + 4, cc, :], pk)
                for jt in range(NJT):
                    pv = psB.tile([P, C], F32, tag="pv", name="pv")
                    sl = slice(jt * P, (jt + 1) * P)
                    nc.tensor.matmul(pv, xr_sb[0][:, sl], wf["v", 0], start=True, stop=False)
                    nc.tensor.matmul(pv, xr_sb[1][:, sl], wf["v", 1], start=False, stop=True)
                    nc.vector.tensor_copy(vT_sb[:, jt // 2, jt % 2, :], pv)

            # ---------- attention (fp8 DoubleRow) ----------
            with (
                tc.tile_pool(name="psS", bufs=2, space="PSUM") as psS,
                tc.tile_pool(name="psAT", bufs=1, space="PSUM") as psAT,
                tc.tile_pool(name="eP", bufs=3) as eP,
                tc.tile_pool(name="oP", bufs=3) as oP,
                tc.tile_pool(name="rP", bufs=2) as rP,
            ):
                for ib in range(NIB):
                    isl = slice(ib * IBLK, (ib + 1) * IBLK)
                    sps = {}

                    def scores(g):
                        # one DoubleRow matmul per 128-key tile: contracts
                        # all 256 channels; two tiles share a 2-bank sp
                        sp = psS.tile([P, 2 * IBLK], F32, tag="sp", name="sp")
                        nc.tensor.matmul(sp[:, 0:IBLK], k_sb[:, 2 * g, :, :],
                                         q_sb[:, :, isl], start=True, stop=True, perf_mode=DR)
                        nc.tensor.matmul(sp[:, IBLK:2 * IBLK], k_sb[:, 2 * g + 1, :, :],
                                         q_sb[:, :, isl], start=True, stop=True, perf_mode=DR)
                        sps[g] = sp

                    aT0 = psAT.tile([P, IBLK], F32, tag="aT0", name="aT0")
                    aT1 = psAT.tile([P, IBLK], F32, tag="aT1", name="aT1")
                    dnb = psAT.tile([P, IBLK], F32, tag="dnb", name="dnb")
                    scores(0)
                    for g in range(NPAIR):
                        if g + 1 < NPAIR:
                            scores(g + 1)
                        eT = eP.tile([P, 2, IBLK], FP8, tag="eT", name="eT")
                        # exp of both key tiles in one 1024-wide ACT op;
                        # shift keeps e^s within fp8 range (softmax-invariant)
                        nc.scalar.activation(out=eT, in_=sps.pop(g),
                                             func=mybir.ActivationFunctionType.Exp,
                                             scale=SCALE, bias=shift_t)
                        st = (g == 0)
                        sp_ = (g == NPAIR - 1)
                        nc.tensor.matmul(aT0, vT_sb[:, g, :, 0:P], eT, start=st, stop=sp_, perf_mode=DR)
                        nc.tensor.matmul(aT1, vT_sb[:, g, :, P:C], eT, start=st, stop=sp_, perf_mode=DR)
                        # all-ones stationary: denominator, broadcast to all
                        # 128 partitions for free
                        nc.tensor.matmul(dnb, ones_pair, eT, start=st, stop=sp_, perf_mode=DR)
                    rec = rP.tile([P, IBLK], F32, tag="rec", name="rec")
                    nc.vector.reciprocal(rec, dnb)
                    for cc, aT in ((0, aT0), (1, aT1)):
                        ot = oP.tile([P, IBLK], F32, tag="ot", name="ot")
                        nc.vector.tensor_mul(ot, aT, rec)
                        nc.sync.dma_start(out=out_d[cc * P:(cc + 1) * P, isl], in_=ot)

    nc.finalize()
    return nc


def _get_program():
    global _PROGRAM
    if _PROGRAM is None:
        _PROGRAM = _build_program()
    return _PROGRAM


def kernel(x, gn_scale, gn_bias, wq, bq, wk, bk, wv, bv, wproj, bproj):
    global LAST_RESULTS
    x = np.asarray(x, dtype=np.float32)
    gn_scale = np.asarray(gn_scale, dtype=np.float32)
    gn_bias = np.asarray(gn_bias, dtype=np.float32)
    wq_ = np.asarray(wq, dtype=np.float32)
    wk_ = np.asarray(wk, dtype=np.float32)
    wv_ = np.asarray(wv, dtype=np.float32)
    wp_ = np.asarray(wproj, dtype=np.float32)
    bq_ = np.asarray(bq, dtype=np.float32)
    bv_ = np.asarray(bv, dtype=np.float32)
    bp_ = np.asarray(bproj, dtype=np.float32)

    b, c, h, w = x.shape
    assert (b, c, h * w) == (B, C, HW), x.shape

    w2 = (wp_.astype(np.float64) @ wv_.astype(np.float64)).astype(np.float32)
    b2h = (wp_.astype(np.float64) @ bv_.astype(np.float64)).astype(np.float32) + bp_

    wqt = np.ascontiguousarray(wq_.T).astype(NPBF16)
    wkt = np.ascontiguousarray(wk_.T).astype(NPBF16)
    w2t = np.ascontiguousarray(w2.T).astype(NPBF16)

    xf = x.reshape(B, C, HW)
    in_maps = []
    for core in range(NCORES):
        bi, hi = core // 2, core % 2
        xi = np.roll(xf[bi], -IH * hi, axis=1)
        in_maps.append({
            "xr": xi.astype(NPBF16),
            "wqt": wqt, "wkt": wkt, "w2t": w2t,
            "bq": bq_, "b2h": b2h,
            "gns": gn_scale, "gnb": gn_bias,
        })

    nc = _get_program()
    res = run_bass_kernel_spmd(nc, in_maps, list(range(NCORES)), trace=TRACE)
    LAST_RESULTS = res

    out = np.empty((B, C, HW), dtype=np.float32)
    for core in range(NCORES):
        bi, hi = core // 2, core % 2
        out[bi][:, hi * IH:(hi + 1) * IH] = (
            res.results[core]["out"] + res.results[core]["b2o"][:, None]
        )
    out += xf
    return out.reshape(B, C, h, w)


# revision 6
# speedup vs baseline: 1.3289x; 1.3289x over previous
"""AttnBlock (GroupNorm + single-head spatial attention + proj + residual)
for Trainium2, SPMD across 8 NeuronCores.

Sharding: data-parallel over batch (4 images) x 2-way split of query
positions per image => 8 cores.  Attention is computed per-image with the
full key/value set on every core, so there are no collectives.

Per-core algorithm (image b, query half h):
  - Spatial positions of the local image copy are rolled so the core's
    2048 query positions are always local positions [0, 2048).  Attention
    and GroupNorm are permutation-invariant over spatial positions, so the
    roll is transparent; the host un-rolls when assembling the output.
  - x is loaded in bf16 (half the HBM traffic of f32); GroupNorm stats on
    bf16 x are within ~1e-5 of exact.  GroupNorm is folded into the
    projections: h = a*x + b, with a folded into the (bf16) weights.
  - wproj is folded into the v projection on the host (softmax rows sum
    to one); the remaining per-channel output constant (w2@b + wproj@bv
    + bproj) is shipped back as a tiny `b2o` output and added on the
    host, together with the residual x.
  - k's projection bias is dropped (adds a softmax-invariant constant).
  - Attention runs in fp8(e4m3) with DoubleRow matmuls (2 fp8 values per
    PE cell => one matmul contracts 256): q/k are packed [128, 2(ch), .],
    so each 128-key score tile is a single matmul; exp is applied to two
    score tiles at once (one 1024-wide ACT op) with a constant shift
    (softmax-invariant, keeps e^s inside fp8 range), writing the packed
    [128, 2(jtile), 512] fp8 moving operand for PV.  PV is computed with
    v^T stationary ([128 j, 2, 128 ch] fp8 pairs) accumulating
    aT[ch, i-block]; an all-ones fp8 stationary produces the softmax
    denominator broadcast across all 128 partitions for free.
  - PE warm-up matmuls fill the initial DMA/stats wait so the HAM clock
    gate is released before the real matmul stream starts.
"""

import numpy as np
import ml_dtypes

import concourse.bacc as bacc
import concourse.bass as bass
import concourse.mybir as mybir
import concourse.tile as tile
from concourse.bass_utils import run_bass_kernel_spmd

F32 = mybir.dt.float32
BF16 = mybir.dt.bfloat16
FP8 = mybir.dt.float8e4
DR = mybir.MatmulPerfMode.DoubleRow

C = 256          # channels
HW = 4096        # spatial positions (64*64)
B = 4            # batch
NCORES = 8
IH = HW // 2     # query positions per core
P = 128          # partitions
NCC = C // P     # channel chunks (2)
IBLK = 512       # query i-block
NIB = IH // IBLK # 4 i-blocks per core
NJT = HW // P    # 32 key tiles
NPAIR = NJT // 2 # 16 key-tile pairs (DoubleRow)
EPS = 1e-6
SCALE = 1.0 / 16.0  # 1/sqrt(C)
SHIFT = 3.0         # exp(s*SCALE - SHIFT): keeps e^s in fp8 e4m3 range

NPBF16 = ml_dtypes.bfloat16

_PROGRAM = None  # cached (nc)
LAST_RESULTS = None  # BassKernelResults of the most recent run (for test harness)
TRACE = False


def _build_program():
    nc = bacc.Bacc()

    xr_d = nc.declare_dram_parameter("xr", [C, HW], BF16, isOutput=False)
    wq_d = nc.declare_dram_parameter("wqt", [C, C], BF16, isOutput=False)
    wk_d = nc.declare_dram_parameter("wkt", [C, C], BF16, isOutput=False)
    w2_d = nc.declare_dram_parameter("w2t", [C, C], BF16, isOutput=False)
    bq_d = nc.declare_dram_parameter("bq", [C], F32, isOutput=False)
    b2h_d = nc.declare_dram_parameter("b2h", [C], F32, isOutput=False)  # wproj@bv+bproj
    gns_d = nc.declare_dram_parameter("gns", [C], F32, isOutput=False)
    gnb_d = nc.declare_dram_parameter("gnb", [C], F32, isOutput=False)
    out_d = nc.declare_dram_parameter("out", [C, IH], F32, isOutput=True)
    b2o_d = nc.declare_dram_parameter("b2o", [C], F32, isOutput=True)

    with tile.TileContext(nc) as tc:
        with (
            tc.tile_pool(name="wt", bufs=1) as wt,
            tc.tile_pool(name="xp", bufs=1) as xp,
            tc.tile_pool(name="qkv", bufs=1) as qkv,
            tc.tile_pool(name="scr", bufs=2) as scr,
        ):
            # ---------- constants ----------
            G = wt.tile([P, P], F32, tag="G", name="G")
            nc.gpsimd.memset(G, 0.0)
            nc.gpsimd.memset(G[0:64, 0:64], 1.0 / 64.0)
            nc.gpsimd.memset(G[64:128, 64:128], 1.0 / 64.0)
            eps_t = wt.tile([P, 1], F32, tag="eps", name="eps")
            nc.vector.memset(eps_t, EPS)
            shift_t = wt.tile([P, 1], F32, tag="shift", name="shift")
            nc.vector.memset(shift_t, -SHIFT)
            ones_pair = wt.tile([P, 2, P], FP8, tag="ones8", name="ones8")
            nc.vector.memset(ones_pair, 1.0)
            ones_row = wt.tile([1, P], BF16, tag="ones_row", name="ones_row")
            nc.vector.memset(ones_row, 1.0)

            # ---------- x loads first (startup critical path) ----------
            # issue from multiple sequencers: one dma_start costs ~0.6us of
            # sequencer issue time, and x is the critical path
            xr_sb = [xp.tile([P, HW], BF16, tag=f"xr{cc}", name=f"xr{cc}")
                     for cc in range(NCC)]
            _eng = [nc.sync, nc.scalar, nc.gpsimd]
            for w in range(8):
                for cc in range(NCC):
                    _eng[(w * NCC + cc) % 3].dma_start(
                        out=xr_sb[cc][:, w * 512:(w + 1) * 512],
                        in_=xr_d[cc * P:(cc + 1) * P, w * 512:(w + 1) * 512],
                    )

            # ---------- load weights / params ----------
            w_sb = {}
            for name, d in (("q", wq_d), ("k", wk_d), ("v", w2_d)):
                for cc in range(NCC):
                    t = wt.tile([P, C], BF16, tag=f"w{name}{cc}", name=f"w{name}{cc}")
                    nc.scalar.dma_start(out=t, in_=d[cc * P:(cc + 1) * P, :])
                    w_sb[name, cc] = t
            par_sb = {}
            for name, d in (("bq", bq_d), ("gns", gns_d), ("gnb", gnb_d)):
                for cc in range(NCC):
                    t = wt.tile([P, 1], F32, tag=f"{name}{cc}", name=f"{name}{cc}")
                    nc.scalar.dma_start(out=t, in_=d[cc * P:(cc + 1) * P].unsqueeze(1))
                    par_sb[name, cc] = t
            b2h_sb = wt.tile([1, C], F32, tag="b2h", name="b2h")
            nc.sync.dma_start(out=b2h_sb, in_=b2h_d[:].unsqueeze(0))

            # ---------- GroupNorm stats (on bf16 x; error ~1e-5) ----------
            with tc.tile_pool(name="psA", bufs=2, space="PSUM") as psA:
                # PE warm-up while x DMA + stats run: fills idle time and
                # brings HAM out of the cold 1.2 GHz state before real work
                warm_ps = psA.tile([P, 128], F32, tag="warm", name="warm")
                warm_rhs = wt.tile([P, 128], F32, tag="warm_rhs", name="warm_rhs")
                nc.gpsimd.memset(warm_rhs, 0.0)
                for _ in range(26):
                    nc.tensor.matmul(warm_ps, G, warm_rhs, start=True, stop=True)
                a_sb, b_sb = [], []
                st6s = [scr.tile([P, 8, 6], F32, tag=f"st6{cc}", name=f"st6{cc}")
                        for cc in range(NCC)]
                for w in range(8):
                    for cc in range(NCC):
                        nc.vector.bn_stats(out=st6s[cc][:, w, :], in_=xr_sb[cc][:, w * 512:(w + 1) * 512])
                for cc in range(NCC):
                    st6 = st6s[cc]
                    mv = scr.tile([P, 2], F32, tag="mv", name="mv")
                    nc.vector.bn_aggr(out=mv, in_=st6)
                    st3 = scr.tile([P, 3], F32, tag="st3", name="st3")
                    nc.vector.tensor_copy(st3[:, 0:2], mv)
                    nc.vector.tensor_mul(st3[:, 2:3], mv[:, 0:1], mv[:, 0:1])
                    gp = psA.tile([P, 3], F32, tag="gp", name="gp")
                    nc.tensor.matmul(gp, G, st3, start=True, stop=True)
                    # group stats, broadcast per channel: mean, E[var], E[mean^2]
                    gs = scr.tile([P, 3], F32, tag="gs", name="gs")
                    nc.vector.tensor_copy(gs, gp)
                    t1 = scr.tile([P, 1], F32, tag="t1", name="t1")
                    nc.vector.tensor_mul(t1, gs[:, 0:1], gs[:, 0:1])
                    vg = scr.tile([P, 1], F32, tag="vg", name="vg")
                    nc.vector.tensor_add(vg, gs[:, 1:2], gs[:, 2:3])
                    nc.vector.tensor_sub(vg, vg, t1)
                    sd = scr.tile([P, 1], F32, tag="sd", name="sd")
                    nc.scalar.activation(out=sd, in_=vg, func=mybir.ActivationFunctionType.Sqrt, bias=eps_t)
                    rstd = scr.tile([P, 1], F32, tag="rstd", name="rstd")
                    nc.vector.reciprocal(rstd, sd)
                    a_t = wt.tile([P, 1], F32, tag=f"a{cc}", name=f"a{cc}")
                    nc.vector.tensor_mul(a_t, rstd, par_sb["gns", cc])
                    t2 = scr.tile([P, 1], F32, tag="t2", name="t2")
                    nc.vector.tensor_mul(t2, gs[:, 0:1], a_t)
                    b_t = wt.tile([P, 1], BF16, tag=f"b{cc}", name=f"b{cc}")
                    nc.vector.tensor_sub(b_t, par_sb["gnb", cc], t2)
                    a_sb.append(a_t)
                    b_sb.append(b_t)

                for _ in range(14):
                    nc.tensor.matmul(warm_ps, G, warm_rhs, start=True, stop=True)

                # ---------- fold GroupNorm scale into weights ----------
                wf = {}
                for name in ("q", "k", "v"):
                    for cc in range(NCC):
                        t = wt.tile([P, C], BF16, tag=f"wf{name}{cc}", name=f"wf{name}{cc}")
                        nc.vector.tensor_scalar_mul(t, w_sb[name, cc], a_sb[cc])
                        wf[name, cc] = t

                # ---------- effective biases ----------
                be = {}
                for cc in range(NCC):
                    bp = psA.tile([P, 1], F32, tag="bp", name="bp")
                    nc.tensor.matmul(bp, w_sb["q", 0][:, cc * P:(cc + 1) * P], b_sb[0], start=True, stop=False)
                    nc.tensor.matmul(bp, w_sb["q", 1][:, cc * P:(cc + 1) * P], b_sb[1], start=False, stop=True)
                    t = wt.tile([P, 1], F32, tag=f"beq{cc}", name=f"beq{cc}")
                    nc.vector.tensor_add(t, bp, par_sb["bq", cc])
                    be["q", cc] = t
                # per-channel output constant w2@b + (wproj@bv + bproj),
                # shipped to the host (softmax rows sum to 1, so it can be
                # added after attention)
                b2p = psA.tile([1, C], F32, tag="b2p", name="b2p")
                nc.tensor.matmul(b2p, b_sb[0], w_sb["v", 0], start=True, stop=False)
                nc.tensor.matmul(b2p, b_sb[1], w_sb["v", 1], start=False, stop=True)
                b2row = wt.tile([1, C], F32, tag="b2row", name="b2row")
                nc.vector.tensor_add(b2row, b2p, b2h_sb)
                nc.sync.dma_start(out=b2o_d[:].unsqueeze(0), in_=b2row)

            # ---------- projections (bf16 in, fp8 packed out) ----------
            # q packed [128, 2(ch chunk), IH]; k packed [128, jt, 2(ch), 128];
            # vT packed [128, jt-pair, 2(jt), C]
            q_sb = qkv.tile([P, NCC, IH], FP8, tag="q8", name="q8")
            k_sb = qkv.tile([P, NJT, NCC, P], FP8, tag="k8", name="k8")
            vT_sb = qkv.tile([P, NPAIR, 2, C], FP8, tag="vT8", name="vT8")

            with tc.tile_pool(name="psB", bufs=3, space="PSUM") as psB:
                for cc in range(NCC):
                    for ib in range(NIB):
                        pq = psB.tile([P, IBLK], F32, tag="pq", name="pq")
                        sl = slice(ib * IBLK, (ib + 1) * IBLK)
                        nc.tensor.matmul(pq, wf["q", 0][:, cc * P:(cc + 1) * P], xr_sb[0][:, sl], start=True, stop=False)
                        nc.tensor.matmul(pq, wf["q", 1][:, cc * P:(cc + 1) * P], xr_sb[1][:, sl], start=False, stop=True)
                        nc.vector.tensor_scalar_add(q_sb[:, cc, sl], pq, be["q", cc])
                for cc in range(NCC):
                    for jb in range(HW // IBLK):
                        pk = psB.tile([P, IBLK], F32, tag="pq", name="pq")
                        sl = slice(jb * IBLK, (jb + 1) * IBLK)
                        nc.tensor.matmul(pk, wf["k", 0][:, cc * P:(cc + 1) * P], xr_sb[0][:, sl], start=True, stop=False)
                        nc.tensor.matmul(pk, wf["k", 1][:, cc * P:(cc + 1) * P], xr_sb[1][:, sl], start=False, stop=True)
                        # k's bias only adds a j-constant to each softmax row
                        # (q_i . bke), so it is dropped; fp8 pack on ACT
                        nc.scalar.copy(k_sb[:, 4 * jb:4 * jb + 4, cc, :], pk)
                for jt in range(NJT):
                    pv = psB.tile([P, C], F32, tag="pv", name="pv")
                    sl = slice(jt * P, (jt + 1) * P)
                    nc.tensor.matmul(pv, xr_sb[0][:, sl], wf["v", 0], start=True, stop=False)
                    nc.tensor.matmul(pv, xr_sb[1][:, sl], wf["v", 1], start=False, stop=True)
                    nc.vector.tensor_copy(vT_sb[:, jt // 2, jt % 2, :], pv)

            # ---------- attention (fp8 DoubleRow) ----------
            with (
                tc.tile_pool(name="psS", bufs=3, space="PSUM") as psS,
                tc.tile_pool(name="psAT", bufs=2, space="PSUM") as psAT,
                tc.tile_pool(name="psDN", bufs=1, space="PSUM") as psDN,
                tc.tile_pool(name="eP", bufs=3) as eP,
                tc.tile_pool(name="oP", bufs=3) as oP,
                tc.tile_pool(name="rP", bufs=2) as rP,
            ):
                for ib in range(NIB):
                    isl = slice(ib * IBLK, (ib + 1) * IBLK)
                    sps = {}
                    eTs = {}

                    def scores(jt):
                        # one DoubleRow matmul per 128-key tile: contracts
                        # all 256 channels in a single pass
                        sp = psS.tile([P, IBLK], F32, tag="sp", name="sp")
                        nc.tensor.matmul(sp, k_sb[:, jt, :, :], q_sb[:, :, isl],
                                         start=True, stop=True, perf_mode=DR)
                        sps[jt] = sp

                    aT0 = psAT.tile([P, IBLK], F32, tag="aT0", name="aT0")
                    aT1 = psAT.tile([P, IBLK], F32, tag="aT1", name="aT1")
                    dnb = psDN.tile([P, IBLK], F32, tag="dnb", name="dnb")
                    scores(0)
                    scores(1)
                    for jt in range(NJT):
                        if jt + 2 < NJT:
                            scores(jt + 2)
                        g, ko = jt // 2, jt % 2
                        if ko == 0:
                            eTs[g] = eP.tile([P, 2, IBLK], FP8, tag="eT", name="eT")
                        # shift keeps e^s within fp8 e4m3 range
                        # (softmax-invariant; un-done by the normalization)
                        nc.scalar.activation(out=eTs[g][:, ko, :], in_=sps.pop(jt),
                                             func=mybir.ActivationFunctionType.Exp,
                                             scale=SCALE, bias=shift_t)
                        if ko == 1:
                            eT = eTs.pop(g)
                            st = (g == 0)
                            sp_ = (g == NPAIR - 1)
                            nc.tensor.matmul(aT0, vT_sb[:, g, :, 0:P], eT, start=st, stop=sp_, perf_mode=DR)
                            nc.tensor.matmul(aT1, vT_sb[:, g, :, P:C], eT, start=st, stop=sp_, perf_mode=DR)
                            # all-ones stationary: denominator, broadcast to
                            # all 128 partitions for free
                            nc.tensor.matmul(dnb, ones_pair, eT, start=st, stop=sp_, perf_mode=DR)
                    rec = rP.tile([P, IBLK], F32, tag="rec", name="rec")
                    nc.vector.reciprocal_approx_fast(out=rec, in_=dnb)
                    for cc, aT in ((0, aT0), (1, aT1)):
                        ot = oP.tile([P, IBLK], F32, tag="ot", name="ot")
                        nc.vector.tensor_mul(ot, aT, rec)
                        nc.sync.dma_start(out=out_d[cc * P:(cc + 1) * P, isl], in_=ot)

    nc.finalize()
    return nc


def _get_program():
    global _PROGRAM
    if _PROGRAM is None:
        _PROGRAM = _build_program()
    return _PROGRAM


def kernel(x, gn_scale, gn_bias, wq, bq, wk, bk, wv, bv, wproj, bproj):
    global LAST_RESULTS
    x = np.asarray(x, dtype=np.float32)
    gn_scale = np.asarray(gn_scale, dtype=np.float32)
    gn_bias = np.asarray(gn_bias, dtype=np.float32)
    wq_ = np.asarray(wq, dtype=np.float32)
    wk_ = np.asarray(wk, dtype=np.float32)
    wv_ = np.asarray(wv, dtype=np.float32)
    wp_ = np.asarray(wproj, dtype=np.float32)
    bq_ = np.asarray(bq, dtype=np.float32)
    bv_ = np.asarray(bv, dtype=np.float32)
    bp_ = np.asarray(bproj, dtype=np.float32)

    b, c, h, w = x.shape
    assert (b, c, h * w) == (B, C, HW), x.shape

    w2 = (wp_.astype(np.float64) @ wv_.astype(np.float64)).astype(np.float32)
    b2h = (wp_.astype(np.float64) @ bv_.astype(np.float64)).astype(np.float32) + bp_

    wqt = np.ascontiguousarray(wq_.T).astype(NPBF16)
    wkt = np.ascontiguousarray(wk_.T).astype(NPBF16)
    w2t = np.ascontiguousarray(w2.T).astype(NPBF16)

    xf = x.reshape(B, C, HW)
    in_maps = []
    for core in range(NCORES):
        bi, hi = core // 2, core % 2
        xi = np.roll(xf[bi], -IH * hi, axis=1)
        in_maps.append({
            "xr": xi.astype(NPBF16),
            "wqt": wqt, "wkt": wkt, "w2t": w2t,
            "bq": bq_, "b2h": b2h,
            "gns": gn_scale, "gnb": gn_bias,
        })

    nc = _get_program()
    res = run_bass_kernel_spmd(nc, in_maps, list(range(NCORES)), trace=TRACE)
    LAST_RESULTS = res

    out = np.empty((B, C, HW), dtype=np.float32)
    for core in range(NCORES):
        bi, hi = core // 2, core % 2
        out[bi][:, hi * IH:(hi + 1) * IH] = (
            res.results[core]["out"] + res.results[core]["b2o"][:, None]
        )
    out += xf
    return out.reshape(B, C, h, w)
